# revision 17
# baseline (speedup 1.0000x reference)
"""Trainium2 Bass kernel for nn_DCM_56040733278668 (dense_cnn).

Data-parallel over batch B=16 across 8 NeuronCores (2 samples/core).

Per-core pipeline (samples s0, s1 packed in partitions [0:64]/[64:128] for
all 64-channel ("mid") tensors):
  A. AdaptiveAvgPool2d(3) of y via strided DVE reduces -> pooled; tiny fp32
     matmuls -> dynamic depthwise weights kpair [128, 9]; two fp8 tap-weight
     banks w2n/w2s [128, 9, 128] = kpair-scaled block-diag (n) and
     anti-block-diag (s = output partitions swapped between samples) W_fi^T.
  B. trans 1x1 (x -> x_in, 256->64) as fp32r matmuls; x_in stored fp8 in a
     zero-padded [128, 106, 106] tile (halo = dilation-5 conv pad).
  C. Per 4-row chunk: each dilated depthwise conv merged with fuse_inside as
     4 fp8 DoubleRow matmuls (tap pairs via strided window APs; 8 taps) per
     dilation; the shared center tap computed once per chunk. d=3 and center
     use the swapped bank so all PSUM->SBUF f-extractions stay partition
     aligned (no partition-moving DMAs). fuse_outside = 2 fp32r x-part
     matmuls + 1 fp8 DoubleRow over [f1;f3] and [f5;c0] k-tiles per
     (sample, out-half); center folded via summed w_fo blocks; b_fi folded
     into b_fo host-side. x DMAs are gated behind y so phase A (which gates
     everything through kpair) gets full HBM bandwidth.
"""

import sys
import numpy as np

sys.path.insert(0, "/opt/trn_rl_repo")

IN_C = 256
MID_C = 64
OUT_C = 256
KS = 3
DILATIONS = (1, 3, 5)
B, H, W = 16, 96, 96
N_CORES = 8
SPC = B // N_CORES  # samples per core = 2
PAD = 5
HP = H + 2 * PAD  # 106
WP = W + 2 * PAD  # 106
CH_ROWS = 4        # rows per compute chunk
PIECE_ROWS = 12    # rows per x DMA piece (3 chunks)
N_PIECES = H // PIECE_ROWS   # 8
Y_ROWS = 24        # rows per y DMA piece
NY_PIECES = H // Y_ROWS      # 4
OST_ROWS = 16      # rows per out staging tile
N_CHUNKS = H // CH_ROWS      # 24

S2 = 256.0   # host scale on wfi2 (keeps fp8 W2 in range)
SF = 16.0    # scale on f tiles (psum * SF/S2)
G = 16.0     # scale on fuse_outside x-part weights; f-part = G/SF = 1

# tap pairs (flat index t = 3*i + j); center tap 4 handled separately
TAP_PAIRS = ((0, 2), (6, 8), (3, 5), (1, 7))

_CACHE = {}


def _tap_off(t, d):
    i, j = t // 3, t % 3
    return d * (i - 1) * WP + d * (j - 1)


def _build(repeat=1, mode="full"):
    import concourse.mybir as mybir
    import concourse.tile as tile
    from concourse import bacc
    import concourse.bass as bass
    import contextlib

    f32 = mybir.dt.float32
    f32r = mybir.dt.float32r
    f16 = mybir.dt.float16
    f8 = mybir.dt.float8e4
    ADD = mybir.AluOpType.add
    MULT = mybir.AluOpType.mult
    DR = mybir.MatmulPerfMode.DoubleRow
    Copy = mybir.ActivationFunctionType.Copy
    Ident = mybir.ActivationFunctionType.Identity

    nc = bacc.Bacc(None, target_bir_lowering=False)

    x = nc.dram_tensor("x", [SPC, IN_C, H, W], f32, kind="ExternalInput")
    y = nc.dram_tensor("y", [SPC, IN_C, H, W], f32, kind="ExternalInput")
    wtr = nc.dram_tensor("wtr", [2, 2, 128, 128], f32, kind="ExternalInput")
    wgk = nc.dram_tensor("wgk", [2, 2, 128, 128], f32, kind="ExternalInput")
    wfi2 = nc.dram_tensor("wfi2", [2, 128, 128], f32, kind="ExternalInput")
    wfox = nc.dram_tensor("wfox", [2, 128, 256], f32, kind="ExternalInput")
    wfoF = nc.dram_tensor("wfoF", [2, 128, 2, 256], f8, kind="ExternalInput")
    wtr8 = nc.dram_tensor("wtr8", [2, 128, 2, 128], f8, kind="ExternalInput")
    btr = nc.dram_tensor("btr", [128, 1], f32, kind="ExternalInput")
    bgk = nc.dram_tensor("bgk", [128, 1], f32, kind="ExternalInput")
    bfo = nc.dram_tensor("bfo", [2, 128, 1], f32, kind="ExternalInput")
    o = nc.dram_tensor("o", [SPC, OUT_C, H, W], f16, kind="ExternalOutput")

    with tile.TileContext(nc) as tc:
        ctx = contextlib.ExitStack()
        with ctx:
            pw = ctx.enter_context(tc.tile_pool(name="pw", bufs=1))
            pbig = ctx.enter_context(tc.tile_pool(name="pbig", bufs=1))
            pw2 = ctx.enter_context(tc.tile_pool(name="pw2", bufs=1))
            ppl = ctx.enter_context(tc.tile_pool(name="ppl", bufs=1))
            pF = ctx.enter_context(tc.tile_pool(name="pF", bufs=6))
            pO = ctx.enter_context(tc.tile_pool(name="pO", bufs=8))
            pxp = ctx.enter_context(tc.tile_pool(name="pxp", bufs=12))
            px8 = ctx.enter_context(tc.tile_pool(name="px8", bufs=6))
            py = ctx.enter_context(tc.tile_pool(name="py", bufs=3))
            psB = ctx.enter_context(tc.tile_pool(name="psB", bufs=2, space="PSUM"))
            psF = ctx.enter_context(tc.tile_pool(name="psF", bufs=3, space="PSUM"))
            psO = ctx.enter_context(tc.tile_pool(name="psO", bufs=3, space="PSUM"))

            # ---------- weights into SBUF (outside the repeat loop) ----------
            wgk_sb = [[pw.tile([128, 128], f32, tag=f"wgk{k}{s}", name=f"wgk{k}{s}")
                       for s in range(2)] for k in range(2)]
            for k in range(2):
                for s in range(2):
                    nc.sync.dma_start(out=wgk_sb[k][s][:], in_=wgk[k, s])
            wfi2_sb = [pw.tile([128, 128], f32, tag=f"wfi2{v}", name=f"wfi2{v}")
                       for v in range(2)]
            for v in range(2):
                nc.sync.dma_start(out=wfi2_sb[v][:], in_=wfi2[v])
            wfox_sb = [pw.tile([128, 256], f32r, tag=f"wfox{k}", name=f"wfox{k}") for k in range(2)]
            for k in range(2):
                nc.sync.dma_start(out=wfox_sb[k][:], in_=wfox[k].bitcast(f32r))
            wfoF_sb = [pw.tile([128, 2, 256], f8, tag=f"wfoF{s}", name=f"wfoF{s}") for s in range(2)]
            for s in range(2):
                nc.sync.dma_start(out=wfoF_sb[s][:], in_=wfoF[s])
            wtr8_sb = [pw.tile([128, 2, 128], f8, tag=f"wtr8{s}", name=f"wtr8{s}") for s in range(2)]
            for s in range(2):
                nc.sync.dma_start(out=wtr8_sb[s][:], in_=wtr8[s])
            btr_sb = pw.tile([128, 1], f32, tag="btr", name="btr")
            nc.sync.dma_start(out=btr_sb[:], in_=btr[:])
            bgk_sb = pw.tile([128, 1], f32, tag="bgk", name="bgk")
            nc.sync.dma_start(out=bgk_sb[:], in_=bgk[:])
            bfo_sb = [pw.tile([128, 1], f32, tag=f"bfo{m}", name=f"bfo{m}") for m in range(2)]
            for m in range(2):
                nc.sync.dma_start(out=bfo_sb[m][:], in_=bfo[m])

            # ---------- big resident tiles ----------
            xin = pbig.tile([128, HP, WP], f8, tag="xin", name="xin")

            # zero halo border of xin (once; interior rewritten every repeat)
            ztop = pw.tile([128, PAD, WP], f8, tag="ztop", name="ztop")
            nc.gpsimd.memset(ztop[:], 0.0)
            zlr = pw.tile([128, H, PAD], f8, tag="zlr", name="zlr")
            nc.gpsimd.memset(zlr[:], 0.0)
            nc.vector.tensor_scalar(out=xin[:, 0:PAD, :], in0=ztop[:], scalar1=1.0, scalar2=None, op0=MULT)
            nc.vector.tensor_scalar(out=xin[:, PAD + H:, :], in0=ztop[:], scalar1=1.0, scalar2=None, op0=MULT)
            nc.vector.tensor_scalar(out=xin[:, PAD:PAD + H, 0:PAD], in0=zlr[:], scalar1=1.0, scalar2=None, op0=MULT)
            nc.vector.tensor_scalar(out=xin[:, PAD:PAD + H, PAD + W:], in0=zlr[:], scalar1=1.0, scalar2=None, op0=MULT)

            rowsum = ppl.tile([128, 4, H, KS], f32, tag="rowsum", name="rowsum")
            pooled = ppl.tile([128, 4, 9], f32, tag="pooled", name="pooled")
            kpair = ppl.tile([128, 9], f32, tag="kpair", name="kpair")
            # fp8 tap-weight banks: normal and sample-swapped output mapping
            w2b = [pw2.tile([128, 9, 128], f8, tag=f"w2b{v}", name=f"w2b{v}")
                   for v in range(2)]

            xin_full = xin[:]
            XPART = xin_full.ap[0]
            XBASE = xin_full.offset

            def xin_ap(off, dims):
                return bass.AP(tensor=xin_full.tensor, offset=XBASE + off,
                               ap=[XPART] + dims)

            w2_full = [w2b[v][:] for v in range(2)]
            WPARTS = [w2_full[v].ap[0] for v in range(2)]

            def w2_ap(v, off, dims):
                return bass.AP(tensor=w2_full[v].tensor,
                               offset=w2_full[v].offset + off,
                               ap=[WPARTS[v]] + dims)

            def body():
                # ---------- phase A: pooling -> kpair -> w2 banks ----------
                y_dmas = []
                for sk in range(4):   # (s, kc)
                    s, kc = sk // 2, sk % 2
                    spans = [(Y_ROWS * p, Y_ROWS) for p in range(NY_PIECES)]
                    if sk == 3:   # split the final piece to shorten the tail
                        spans = spans[:-1] + [(72, 12), (84, 12)]
                    for (a, n) in spans:
                        yp = py.tile([128, Y_ROWS, W], f32, tag="ypc", name="ypc")
                        ydma = nc.sync.dma_start(
                            out=yp[:, 0:n, :],
                            in_=y[s, 128 * kc:128 * (kc + 1), a:a + n, :])
                        y_dmas.append(ydma)
                        nc.vector.tensor_reduce(
                            out=rowsum[:, sk, a:a + n, :],
                            in_=yp[:, 0:n, :].rearrange("p r (j w) -> p r j w", j=KS),
                            axis=mybir.AxisListType.X, op=ADD)
                    nc.vector.tensor_reduce(
                        out=pooled[:, sk, :],
                        in_=rowsum[:, sk].rearrange("p (hb h) j -> p hb j h", h=H // KS),
                        axis=mybir.AxisListType.X, op=ADD)
                kp = psO.tile([128, 9], f32, tag="ops", name="kpsum")
                for sk in range(4):
                    s, kc = sk // 2, sk % 2
                    nc.tensor.matmul(kp[:], wgk_sb[kc][s][:], pooled[:, sk, :],
                                     start=(sk == 0), stop=(sk == 3))
                nc.vector.tensor_scalar(out=kpair[:], in0=kp[:],
                                        scalar1=1.0 / ((H // KS) * (W // KS)),
                                        scalar2=bgk_sb[:], op0=MULT, op1=ADD)
                build_order = [(0, t) for pr in TAP_PAIRS for t in pr]
                build_order += [(1, t) for pr in TAP_PAIRS for t in pr]
                build_order += [(1, 4)]
                for i, (v, t) in enumerate(build_order):
                    if mode == "nopool":
                        sc = 0.01
                    else:
                        sc = kpair[:, t:t + 1]
                    if i % 2 == 0:
                        nc.vector.tensor_scalar(out=w2b[v][:, t, :],
                                                in0=wfi2_sb[v][:], scalar1=sc,
                                                scalar2=None, op0=MULT)
                    else:
                        nc.scalar.activation(out=w2b[v][:, t, :],
                                             in_=wfi2_sb[v][:], func=Copy,
                                             scale=sc)

                # ---------- phases B + C interleaved ----------
                xpieces = {}

                import bass_rust as _br

                def emit_piece(p):
                    r0 = PIECE_ROWS * p
                    xps = []
                    for sk in range(4):
                        s, kc = sk // 2, sk % 2
                        xp_t = pxp.tile([128, PIECE_ROWS, W], f32r, tag="xpc", name="xpc")
                        xdma = nc.sync.dma_start(
                            out=xp_t[:],
                            in_=x[s, 128 * kc:128 * (kc + 1), r0:r0 + PIECE_ROWS, :].bitcast(f32r))
                        if p < 2:
                            gate = y_dmas[-3 if p == 0 else -1]
                            _br.add_dep_helper(xdma.ins, gate.ins, reason="pace x behind y")
                        xps.append(xp_t)
                    xpieces[p] = xps
                    # fp8 copies of x for the trans DoubleRow matmuls
                    x8s = []
                    for s in range(2):
                        x8_t = px8.tile([128, 2, PIECE_ROWS, W], f8, tag="x8", name="x8")
                        for kc in range(2):
                            if (s + kc) % 2 == 0:
                                nc.vector.tensor_scalar(out=x8_t[:, kc], in0=xps[2 * s + kc][:],
                                                        scalar1=1.0, scalar2=None, op0=MULT)
                            else:
                                nc.scalar.activation(out=x8_t[:, kc], in_=xps[2 * s + kc][:],
                                                     func=Copy, scale=1.0)
                        x8s.append(x8_t)
                    for third in range(PIECE_ROWS // CH_ROWS):
                        rr = third * CH_ROWS
                        pt = psB.tile([128, CH_ROWS, W], f32, tag="ptrans", name="ptrans")
                        for s in range(2):
                            nc.tensor.matmul(pt[:], wtr8_sb[s][:],
                                             x8s[s][:, :, rr:rr + CH_ROWS, :],
                                             start=(s == 0), stop=(s == 1),
                                             perf_mode=DR)
                        xin_dst = xin[:, PAD + r0 + rr:PAD + r0 + rr + CH_ROWS, PAD:PAD + W]
                        if (3 * p + third) % 2 == 0:
                            nc.vector.tensor_scalar(out=xin_dst, in0=pt[:],
                                                    scalar1=1.0 / 64.0, scalar2=btr_sb[:],
                                                    op0=MULT, op1=ADD)
                        else:
                            nc.scalar.activation(out=xin_dst, in_=pt[:], func=Ident,
                                                 bias=btr_sb[:], scale=1.0 / 64.0)

                ost = {}
                fstore = {}

                def emit_dw(c):
                    r0 = CH_ROWS * c
                    base = (PAD + r0) * WP + PAD
                    Ffo = [pF.tile([128, 2, CH_ROWS, W], f8, tag="ffo", name=f"ffo{s}")
                           for s in range(2)]

                    def copy_pair(i, dst_a, src_a, dst_b, src_b):
                        # alternate engines; emit right after the source psum
                        # group stops so its bank recycles ASAP
                        if i % 2 == 0:
                            nc.vector.tensor_scalar(out=dst_a, in0=src_a,
                                                    scalar1=SF / S2, scalar2=None,
                                                    op0=MULT)
                            nc.scalar.activation(out=dst_b, in_=src_b, func=Copy,
                                                 scale=SF / S2)
                        else:
                            nc.scalar.activation(out=dst_a, in_=src_a, func=Copy,
                                                 scale=SF / S2)
                            nc.vector.tensor_scalar(out=dst_b, in0=src_b,
                                                    scalar1=SF / S2, scalar2=None,
                                                    op0=MULT)

                    def one_dil(d):
                        v = 1 if d == 3 else 0   # swapped bank for d=3
                        fp = psF.tile([128, CH_ROWS, W], f32, tag="fps", name="fps")
                        pairs = (TAP_PAIRS[:1] if mode == "nodw" else TAP_PAIRS)
                        for pi, (ta, tb) in enumerate(pairs):
                            rhs = xin_ap(base + _tap_off(ta, d),
                                         [[_tap_off(tb, d) - _tap_off(ta, d), 2],
                                          [WP, CH_ROWS], [1, W]])
                            lhsT = w2_ap(v, ta * 128, [[(tb - ta) * 128, 2], [1, 128]])
                            nc.tensor.matmul(fp[:], lhsT, rhs, start=(pi == 0),
                                             stop=(pi == len(pairs) - 1),
                                             perf_mode=DR)
                        # partition-aligned extractions -> fp8 f tiles (x SF/S2)
                        if d == 1:
                            copy_pair(0, Ffo[0][0:64, 0], fp[0:64],
                                      Ffo[1][64:128, 0], fp[64:128])
                        elif d == 3:   # swapped: s1 in low half
                            copy_pair(1, Ffo[1][0:64, 0], fp[0:64],
                                      Ffo[0][64:128, 0], fp[64:128])
                        else:
                            copy_pair(0, Ffo[0][0:64, 1], fp[0:64],
                                      Ffo[1][64:128, 1], fp[64:128])

                    def part1():
                        one_dil(1)
                        one_dil(3)

                    def part2():
                        one_dil(5)
                        # center tap (shared by all dilations), swapped bank
                        fc = psF.tile([128, CH_ROWS, W], f32, tag="fps", name="fpc")
                        nc.tensor.matmul(fc[:], w2b[1][:, 4, :],
                                         xin_ap(base, [[WP, CH_ROWS], [1, W]]),
                                         start=True, stop=True)
                        copy_pair(1, Ffo[1][0:64, 1], fc[0:64],
                                  Ffo[0][64:128, 1], fc[64:128])
                        fstore[c] = Ffo
                    return part1, part2

                def emit_fo(c, samples=(0, 1)):
                    r0 = CH_ROWS * c
                    Ffo = fstore[c]
                    p, rr0 = c // (PIECE_ROWS // CH_ROWS), (c % (PIECE_ROWS // CH_ROWS)) * CH_ROWS
                    xps = xpieces[p]
                    if c % 4 == 0 and samples[0] == 0:
                        for key in ((0, 0), (0, 1), (1, 0), (1, 1)):
                            ost[key] = pO.tile([128, OST_ROWS, W], f16, tag="ost", name="ost")
                    for s in samples:
                        for mj in range(2):
                            po = psO.tile([128, CH_ROWS, W], f32, tag="ops", name="ops")
                            if mode == "nofo":
                                nc.tensor.matmul(po[:], wfox_sb[0][:, 128 * mj:128 * (mj + 1)],
                                                 xps[2 * s][:, rr0:rr0 + CH_ROWS, :],
                                                 start=True, stop=True)
                            else:
                                for kc in range(2):
                                    nc.tensor.matmul(po[:], wfox_sb[kc][:, 128 * mj:128 * (mj + 1)],
                                                     xps[2 * s + kc][:, rr0:rr0 + CH_ROWS, :],
                                                     start=(kc == 0), stop=False)
                                nc.tensor.matmul(po[:],
                                                 wfoF_sb[s][:, :, 128 * mj:128 * (mj + 1)],
                                                 Ffo[s][:], start=False, stop=True,
                                                 perf_mode=DR)
                            stg = ost[(s, mj)]
                            rr = (c % 4) * CH_ROWS
                            if (s + mj) % 2 == 0:
                                nc.vector.tensor_scalar(out=stg[:, rr:rr + CH_ROWS, :], in0=po[:],
                                                        scalar1=1.0 / G, scalar2=bfo_sb[mj][:],
                                                        op0=MULT, op1=ADD)
                            else:
                                nc.scalar.activation(out=stg[:, rr:rr + CH_ROWS, :], in_=po[:],
                                                     func=Ident, bias=bfo_sb[mj][:],
                                                     scale=1.0 / G)
                    if samples[-1] != 1:
                        return
                    if c >= N_CHUNKS - 4 and c % 2 == 1:
                        # tail: flush every 8 rows so the last DMA starts sooner
                        rr = ((c % 4) // 2) * (OST_ROWS // 2)
                        for s in range(2):
                            for mj in range(2):
                                nc.gpsimd.dma_start(
                                    out=o[s, 128 * mj:128 * (mj + 1), r0 + CH_ROWS - OST_ROWS // 2:r0 + CH_ROWS, :],
                                    in_=ost[(s, mj)][:, rr:rr + OST_ROWS // 2, :])
                    elif c % 4 == 3:
                        for s in range(2):
                            for mj in range(2):
                                nc.gpsimd.dma_start(
                                    out=o[s, 128 * mj:128 * (mj + 1), r0 + CH_ROWS - OST_ROWS:r0 + CH_ROWS, :],
                                    in_=ost[(s, mj)][:])

                # pieces of 3 chunks; chunks lag one piece; fo of chunk c-1
                # interleaves between the dilation halves of dw chunk c
                emitted_dw = 0
                emitted_fo = 0

                def step_dw(c):
                    p1, p2 = emit_dw(c)
                    p1()
                    if c >= 1:
                        emit_fo(c - 1, samples=(0,))
                    p2()
                    if c >= 1:
                        emit_fo(c - 1, samples=(1,))
                        fstore.pop(c - 1)

                for p in range(N_PIECES):
                    emit_piece(p)
                    hi = 3 * p + 1  # chunk c needs xin rows through 4c+9
                    while emitted_dw < hi:
                        step_dw(emitted_dw)
                        emitted_dw += 1
                while emitted_dw < N_CHUNKS:
                    step_dw(emitted_dw)
                    emitted_dw += 1
                emit_fo(N_CHUNKS - 1)
                fstore.pop(N_CHUNKS - 1)

            if repeat == 1:
                body()
            else:
                with tc.For_i(0, repeat, 1):
                    body()

    nc.compile()
    return nc


def _prep_weights(w_gk, b_gk, w_tr, b_tr, w_fi, b_fi, w_fo, b_fo):
    import ml_dtypes
    f32 = np.float32
    f8 = ml_dtypes.float8_e4m3
    wtr = np.zeros((2, 2, 128, 128), f32)
    wgk = np.zeros((2, 2, 128, 128), f32)
    for kc in range(2):
        blkT = w_tr[:, 128 * kc:128 * (kc + 1)].T  # [128 in, 64 mid]
        blkG = w_gk[:, 128 * kc:128 * (kc + 1)].T
        for s in range(2):
            wtr[kc, s, :, 64 * s:64 * (s + 1)] = blkT
            wgk[kc, s, :, 64 * s:64 * (s + 1)] = blkG
    wfi2 = np.zeros((2, 128, 128), f32)
    wfi2[0, 0:64, 0:64] = w_fi.T * S2
    wfi2[0, 64:128, 64:128] = w_fi.T * S2
    wfi2[1, 0:64, 64:128] = w_fi.T * S2
    wfi2[1, 64:128, 0:64] = w_fi.T * S2
    # fuse_outside: cat = [x(0:256), f1(256:320), f3(320:384), f5(384:448)]
    wfox = np.zeros((2, 128, 256), f32)
    for kc in range(2):
        wfox[kc] = w_fo[:, 128 * kc:128 * (kc + 1)].T * G
    b1 = w_fo[:, 256:320].T   # [64, 256]
    b3 = w_fo[:, 320:384].T
    b5 = w_fo[:, 384:448].T
    bc = b1 + b3 + b5
    wfoF = np.zeros((2, 128, 2, 256), f32)
    wfoF[0, 0:64, 0] = b1
    wfoF[0, 64:128, 0] = b3
    wfoF[0, 0:64, 1] = b5
    wfoF[0, 64:128, 1] = bc
    wfoF[1, 0:64, 0] = b3
    wfoF[1, 64:128, 0] = b1
    wfoF[1, 0:64, 1] = bc
    wfoF[1, 64:128, 1] = b5
    wfoF = (wfoF * (G / SF)).astype(f8)
    wtr8 = np.zeros((2, 128, 2, 128), f32)
    for s in range(2):
        for kc in range(2):
            wtr8[s, :, kc, 64 * s:64 * (s + 1)] = w_tr[:, 128 * kc:128 * (kc + 1)].T * 64.0
    wtr8 = wtr8.astype(f8)
    btr = np.tile(b_tr, 2).reshape(128, 1).astype(f32)
    bgk = np.tile(b_gk, 2).reshape(128, 1).astype(f32)
    bfo_t = b_fo + w_fo[:, 256:448] @ np.tile(b_fi, 3)
    bfo = bfo_t.reshape(2, 128, 1).astype(f32)
    return dict(wtr=wtr, wgk=wgk, wfi2=wfi2, wfox=wfox, wfoF=wfoF, wtr8=wtr8,
                btr=btr, bgk=bgk, bfo=bfo)


def _get_nc(repeat=1, mode="full"):
    key = ("nc", repeat, mode)
    if key not in _CACHE:
        _CACHE[key] = _build(repeat, mode)
    return _CACHE[key]


def _in_maps(x, y, wd):
    in_maps = []
    for c in range(N_CORES):
        m = dict(wd)
        m["x"] = np.ascontiguousarray(x[SPC * c:SPC * (c + 1)])
        m["y"] = np.ascontiguousarray(y[SPC * c:SPC * (c + 1)])
        in_maps.append(m)
    return in_maps


def kernel(x, y, w_gk, b_gk, w_tr, b_tr, w_fi, b_fi, w_fo, b_fo):
    from concourse.bass_utils import run_bass_kernel_spmd

    nc = _get_nc(1)
    wd = _prep_weights(
        np.asarray(w_gk, np.float32), np.asarray(b_gk, np.float32),
        np.asarray(w_tr, np.float32), np.asarray(b_tr, np.float32),
        np.asarray(w_fi, np.float32), np.asarray(b_fi, np.float32),
        np.asarray(w_fo, np.float32), np.asarray(b_fo, np.float32))
    in_maps = _in_maps(np.asarray(x, np.float32), np.asarray(y, np.float32), wd)
    res = run_bass_kernel_spmd(nc, in_maps, core_ids=list(range(N_CORES)))
    out = np.concatenate([np.asarray(res.results[c]["o"], np.float32)
                          for c in range(N_CORES)], axis=0)
    return out


# ---------------- timing (dev-only; not used by the grader) ----------------

def _make_callable(nc):
    import jax
    import concourse.mybir as mybir
    from concourse.bass2jax import _bass_exec_p, partition_id_tensor
    from jax.sharding import Mesh, PartitionSpec
    from jax.experimental.shard_map import shard_map

    in_names, out_names, out_avals = [], [], []
    for alloc in nc.m.functions[0].allocations:
        if not isinstance(alloc, mybir.MemoryLocationSet):
            continue
        name = alloc.memorylocations[0].name
        if alloc.kind == "ExternalInput":
            if nc.partition_id_tensor is None or name != nc.partition_id_tensor.name:
                in_names.append(name)
        elif alloc.kind == "ExternalOutput":
            out_names.append(name)
            out_avals.append(jax.core.ShapedArray(tuple(alloc.tensor_shape),
                                                  mybir.dt.np(alloc.dtype)))
    n_params = len(in_names)
    all_in = list(in_names) + list(out_names)
    part = nc.partition_id_tensor.name if nc.partition_id_tensor else None
    if part:
        all_in.append(part)

    def _body(*args):
        operands = list(args)
        if part:
            operands.append(partition_id_tensor())
        outs = _bass_exec_p.bind(
            *operands, out_avals=tuple(out_avals), in_names=tuple(all_in),
            out_names=tuple(out_names), lowering_input_output_aliases=(),
            sim_require_finite=True, sim_require_nnan=True, nc=nc)
        return tuple(outs)

    devices = jax.devices()[:N_CORES]
    mesh = Mesh(np.asarray(devices), ("core",))
    nin = n_params + len(out_names)
    fn = jax.jit(shard_map(_body, mesh=mesh, in_specs=(PartitionSpec("core"),) * nin,
                           out_specs=(PartitionSpec("core"),) * len(out_names),
                           check_rep=False), keep_unused=True)
    return fn, in_names, out_names, out_avals


def _prep_fn(repeat, in_maps, mode="full"):
    import jax
    nc = _get_nc(repeat, mode)
    fn, in_names, out_names, out_avals = _make_callable(nc)
    concat_in = []
    for n in in_names:
        per = [np.asarray(in_maps[c][n]) for c in range(N_CORES)]
        concat_in.append(np.concatenate(per, axis=0))
    zeros = [np.zeros((N_CORES * a.shape[0], *a.shape[1:]), a.dtype) for a in out_avals]
    dev_in = [jax.device_put(a) for a in concat_in] + [jax.device_put(z) for z in zeros]
    return fn, dev_in


def _time_pair(in_maps, R=33, rounds=16, mode="full"):
    """Interleaved timing of the R=1 and R=R variants so host/tunnel drift
    cancels. Returns (t1_min, tR_min)."""
    import jax, time
    fn1, in1 = _prep_fn(1, in_maps, mode)
    fnR, inR = _prep_fn(R, in_maps, mode)
    for _ in range(3):
        jax.block_until_ready(fn1(*in1))
        jax.block_until_ready(fnR(*inR))
    t1s, tRs = [], []
    for _ in range(rounds):
        t0 = time.perf_counter()
        jax.block_until_ready(fn1(*in1))
        t1s.append(time.perf_counter() - t0)
        t0 = time.perf_counter()
        jax.block_until_ready(fnR(*inR))
        tRs.append(time.perf_counter() - t0)
    return min(t1s), min(tRs)


def measure_exec_ns(R=33, trials=16, mode="full"):
    rng = np.random.default_rng(0)
    wd = _prep_weights(
        rng.standard_normal((64, 256)).astype(np.float32) * 0.06,
        rng.standard_normal(64).astype(np.float32) * 0.06,
        rng.standard_normal((64, 256)).astype(np.float32) * 0.06,
        rng.standard_normal(64).astype(np.float32) * 0.06,
        rng.standard_normal((64, 64)).astype(np.float32) * 0.12,
        rng.standard_normal(64).astype(np.float32) * 0.12,
        rng.standard_normal((256, 448)).astype(np.float32) * 0.05,
        rng.standard_normal(256).astype(np.float32) * 0.05)
    x = rng.standard_normal((B, IN_C, H, W)).astype(np.float32)
    y = rng.standard_normal((B, IN_C, H, W)).astype(np.float32)
    in_maps = _in_maps(x, y, wd)
    t1, tR = _time_pair(in_maps, R=R, rounds=trials, mode=mode)
    per_iter = (tR - t1) / (R - 1)
    print(f"t1={t1*1e3:.3f} ms  t{R}={tR*1e3:.3f} ms  per-iter={per_iter*1e6:.1f} us")
    return per_iter * 1e9


# revision 18
# speedup vs baseline: 1.1399x; 1.1399x over previous
"""Trainium2 Bass kernel for nn_DCM_56040733278668 (dense_cnn).

Data-parallel over batch B=16 across 8 NeuronCores (2 samples/core).

Per-core pipeline (samples s0, s1 packed in partitions [0:64]/[64:128] for
all 64-channel ("mid") tensors):
  A. AdaptiveAvgPool2d(3) of y via strided DVE reduces -> pooled; tiny fp32
     matmuls -> dynamic depthwise weights kpair [128, 9]; two fp8 tap-weight
     banks w2n/w2s [128, 9, 128] = kpair-scaled block-diag (n) and
     anti-block-diag (s = output partitions swapped between samples) W_fi^T.
  B. trans 1x1 (x -> x_in, 256->64) as fp32r matmuls; x_in stored fp8 in a
     zero-padded [128, 106, 106] tile (halo = dilation-5 conv pad).
  C. Per 4-row chunk: each dilated depthwise conv merged with fuse_inside as
     4 fp8 DoubleRow matmuls (tap pairs via strided window APs; 8 taps) per
     dilation; the shared center tap computed once per chunk. d=3 and center
     use the swapped bank so all PSUM->SBUF f-extractions stay partition
     aligned (no partition-moving DMAs). fuse_outside = 2 fp32r x-part
     matmuls + 1 fp8 DoubleRow over [f1;f3] and [f5;c0] k-tiles per
     (sample, out-half); center folded via summed w_fo blocks; b_fi folded
     into b_fo host-side. x DMAs are gated behind y so phase A (which gates
     everything through kpair) gets full HBM bandwidth.
"""

import sys
import numpy as np

sys.path.insert(0, "/opt/trn_rl_repo")

IN_C = 256
MID_C = 64
OUT_C = 256
KS = 3
DILATIONS = (1, 3, 5)
B, H, W = 16, 96, 96
N_CORES = 8
SPC = B // N_CORES  # samples per core = 2
PAD = 5
HP = H + 2 * PAD  # 106
WP = W + 2 * PAD  # 106
CH_ROWS = 4        # rows per compute chunk
PIECE_ROWS = 12    # rows per x DMA piece (3 chunks)
N_PIECES = H // PIECE_ROWS   # 8
Y_ROWS = 24        # rows per y DMA piece
NY_PIECES = H // Y_ROWS      # 4
OST_ROWS = 16      # rows per out staging tile
N_CHUNKS = H // CH_ROWS      # 24

S2 = 256.0   # host scale on wfi2 (keeps fp8 W2 in range)
SF = 16.0    # scale on f tiles (psum * SF/S2)
G = 16.0     # scale on fuse_outside x-part weights; f-part = G/SF = 1

# tap pairs (flat index t = 3*i + j); center tap 4 handled separately
TAP_PAIRS = ((0, 2), (6, 8), (3, 5), (1, 7))

_CACHE = {}


def _tap_off(t, d):
    i, j = t // 3, t % 3
    return d * (i - 1) * WP + d * (j - 1)


def _build(repeat=1, mode="full"):
    import concourse.mybir as mybir
    import concourse.tile as tile
    from concourse import bacc
    import concourse.bass as bass
    import contextlib

    f32 = mybir.dt.float32
    f32r = mybir.dt.float32r
    f16 = mybir.dt.float16
    f8 = mybir.dt.float8e4
    ADD = mybir.AluOpType.add
    MULT = mybir.AluOpType.mult
    DR = mybir.MatmulPerfMode.DoubleRow
    Copy = mybir.ActivationFunctionType.Copy
    Ident = mybir.ActivationFunctionType.Identity

    nc = bacc.Bacc(None, target_bir_lowering=False)

    x = nc.dram_tensor("x", [SPC, IN_C, H, W], f32, kind="ExternalInput")
    y = nc.dram_tensor("y", [SPC, IN_C, H, W], f32, kind="ExternalInput")
    wtr = nc.dram_tensor("wtr", [2, 2, 128, 128], f32, kind="ExternalInput")
    wgk = nc.dram_tensor("wgk", [2, 2, 128, 128], f32, kind="ExternalInput")
    wfi2 = nc.dram_tensor("wfi2", [2, 128, 128], f32, kind="ExternalInput")
    wfox = nc.dram_tensor("wfox", [2, 128, 256], f32, kind="ExternalInput")
    wfoF = nc.dram_tensor("wfoF", [2, 128, 2, 256], f8, kind="ExternalInput")
    wtr8 = nc.dram_tensor("wtr8", [2, 128, 2, 128], f8, kind="ExternalInput")
    btr = nc.dram_tensor("btr", [128, 1], f32, kind="ExternalInput")
    bgk = nc.dram_tensor("bgk", [128, 1], f32, kind="ExternalInput")
    bfo = nc.dram_tensor("bfo", [2, 128, 1], f32, kind="ExternalInput")
    o = nc.dram_tensor("o", [SPC, OUT_C, H, W], f16, kind="ExternalOutput")

    with tile.TileContext(nc) as tc:
        ctx = contextlib.ExitStack()
        with ctx:
            pw = ctx.enter_context(tc.tile_pool(name="pw", bufs=1))
            pbig = ctx.enter_context(tc.tile_pool(name="pbig", bufs=1))
            pw2 = ctx.enter_context(tc.tile_pool(name="pw2", bufs=1))
            ppl = ctx.enter_context(tc.tile_pool(name="ppl", bufs=1))
            pF = ctx.enter_context(tc.tile_pool(name="pF", bufs=6))
            pO = ctx.enter_context(tc.tile_pool(name="pO", bufs=8))
            pxp = ctx.enter_context(tc.tile_pool(name="pxp", bufs=12))
            px8 = ctx.enter_context(tc.tile_pool(name="px8", bufs=6))
            py = ctx.enter_context(tc.tile_pool(name="py", bufs=3))
            psB = ctx.enter_context(tc.tile_pool(name="psB", bufs=2, space="PSUM"))
            psF = ctx.enter_context(tc.tile_pool(name="psF", bufs=3, space="PSUM"))
            psO = ctx.enter_context(tc.tile_pool(name="psO", bufs=3, space="PSUM"))

            # ---------- weights into SBUF (outside the repeat loop) ----------
            wgk_sb = [[pw.tile([128, 128], f32, tag=f"wgk{k}{s}", name=f"wgk{k}{s}")
                       for s in range(2)] for k in range(2)]
            for k in range(2):
                for s in range(2):
                    nc.sync.dma_start(out=wgk_sb[k][s][:], in_=wgk[k, s])
            wfi2_sb = [pw.tile([128, 128], f32, tag=f"wfi2{v}", name=f"wfi2{v}")
                       for v in range(2)]
            for v in range(2):
                nc.sync.dma_start(out=wfi2_sb[v][:], in_=wfi2[v])
            wfox_sb = [pw.tile([128, 256], f32r, tag=f"wfox{k}", name=f"wfox{k}") for k in range(2)]
            for k in range(2):
                nc.sync.dma_start(out=wfox_sb[k][:], in_=wfox[k].bitcast(f32r))
            wfoF_sb = [pw.tile([128, 2, 256], f8, tag=f"wfoF{s}", name=f"wfoF{s}") for s in range(2)]
            for s in range(2):
                nc.sync.dma_start(out=wfoF_sb[s][:], in_=wfoF[s])
            wtr8_sb = [pw.tile([128, 2, 128], f8, tag=f"wtr8{s}", name=f"wtr8{s}") for s in range(2)]
            for s in range(2):
                nc.sync.dma_start(out=wtr8_sb[s][:], in_=wtr8[s])
            btr_sb = pw.tile([128, 1], f32, tag="btr", name="btr")
            nc.sync.dma_start(out=btr_sb[:], in_=btr[:])
            bgk_sb = pw.tile([128, 1], f32, tag="bgk", name="bgk")
            nc.sync.dma_start(out=bgk_sb[:], in_=bgk[:])
            bfo_sb = [pw.tile([128, 1], f32, tag=f"bfo{m}", name=f"bfo{m}") for m in range(2)]
            for m in range(2):
                nc.sync.dma_start(out=bfo_sb[m][:], in_=bfo[m])

            # ---------- big resident tiles ----------
            xin = pbig.tile([128, HP, WP], f8, tag="xin", name="xin")

            # zero halo border of xin (once; interior rewritten every repeat)
            ztop = pw.tile([128, PAD, WP], f8, tag="ztop", name="ztop")
            nc.gpsimd.memset(ztop[:], 0.0)
            zlr = pw.tile([128, H, PAD], f8, tag="zlr", name="zlr")
            nc.gpsimd.memset(zlr[:], 0.0)
            nc.vector.tensor_scalar(out=xin[:, 0:PAD, :], in0=ztop[:], scalar1=1.0, scalar2=None, op0=MULT)
            nc.vector.tensor_scalar(out=xin[:, PAD + H:, :], in0=ztop[:], scalar1=1.0, scalar2=None, op0=MULT)
            nc.vector.tensor_scalar(out=xin[:, PAD:PAD + H, 0:PAD], in0=zlr[:], scalar1=1.0, scalar2=None, op0=MULT)
            nc.vector.tensor_scalar(out=xin[:, PAD:PAD + H, PAD + W:], in0=zlr[:], scalar1=1.0, scalar2=None, op0=MULT)

            rowsum = ppl.tile([128, 4, H, KS], f32, tag="rowsum", name="rowsum")
            pooled = ppl.tile([128, 4, 9], f32, tag="pooled", name="pooled")
            kpair = ppl.tile([128, 9], f32, tag="kpair", name="kpair")
            # fp8 tap-weight banks: normal and sample-swapped output mapping
            w2b = [pw2.tile([128, 9, 128], f8, tag=f"w2b{v}", name=f"w2b{v}")
                   for v in range(2)]

            xin_full = xin[:]
            XPART = xin_full.ap[0]
            XBASE = xin_full.offset

            def xin_ap(off, dims):
                return bass.AP(tensor=xin_full.tensor, offset=XBASE + off,
                               ap=[XPART] + dims)

            w2_full = [w2b[v][:] for v in range(2)]
            WPARTS = [w2_full[v].ap[0] for v in range(2)]

            def w2_ap(v, off, dims):
                return bass.AP(tensor=w2_full[v].tensor,
                               offset=w2_full[v].offset + off,
                               ap=[WPARTS[v]] + dims)

            def body():
                # ---------- phase A: pooling -> kpair -> w2 banks ----------
                y_dmas = []
                for sk in range(4):   # (s, kc)
                    s, kc = sk // 2, sk % 2
                    spans = [(Y_ROWS * p, Y_ROWS) for p in range(NY_PIECES)]
                    if sk == 3:   # split the final piece to shorten the tail
                        spans = spans[:-1] + [(72, 12), (84, 12)]
                    for (a, n) in spans:
                        yp = py.tile([128, Y_ROWS, W], f32, tag="ypc", name="ypc")
                        ydma = nc.sync.dma_start(
                            out=yp[:, 0:n, :],
                            in_=y[s, 128 * kc:128 * (kc + 1), a:a + n, :])
                        y_dmas.append(ydma)
                        nc.vector.tensor_reduce(
                            out=rowsum[:, sk, a:a + n, :],
                            in_=yp[:, 0:n, :].rearrange("p r (j w) -> p r j w", j=KS),
                            axis=mybir.AxisListType.X, op=ADD)
                    nc.vector.tensor_reduce(
                        out=pooled[:, sk, :],
                        in_=rowsum[:, sk].rearrange("p (hb h) j -> p hb j h", h=H // KS),
                        axis=mybir.AxisListType.X, op=ADD)
                kp = psO.tile([128, 9], f32, tag="ops", name="kpsum")
                for sk in range(4):
                    s, kc = sk // 2, sk % 2
                    nc.tensor.matmul(kp[:], wgk_sb[kc][s][:], pooled[:, sk, :],
                                     start=(sk == 0), stop=(sk == 3))
                nc.vector.tensor_scalar(out=kpair[:], in0=kp[:],
                                        scalar1=1.0 / ((H // KS) * (W // KS)),
                                        scalar2=bgk_sb[:], op0=MULT, op1=ADD)
                build_order = [(0, t) for pr in TAP_PAIRS for t in pr]
                build_order += [(1, t) for pr in TAP_PAIRS for t in pr]
                build_order += [(1, 4)]
                for i, (v, t) in enumerate(build_order):
                    if mode == "nopool":
                        sc = 0.01
                    else:
                        sc = kpair[:, t:t + 1]
                    if i % 2 == 0:
                        nc.vector.tensor_scalar(out=w2b[v][:, t, :],
                                                in0=wfi2_sb[v][:], scalar1=sc,
                                                scalar2=None, op0=MULT)
                    else:
                        nc.scalar.activation(out=w2b[v][:, t, :],
                                             in_=wfi2_sb[v][:], func=Copy,
                                             scale=sc)

                # ---------- phases B + C interleaved ----------
                xpieces = {}

                import bass_rust as _br

                def emit_piece(p):
                    r0 = PIECE_ROWS * p
                    xps = []
                    for sk in range(4):
                        s, kc = sk // 2, sk % 2
                        xp_t = pxp.tile([128, PIECE_ROWS, W], f32r, tag="xpc", name="xpc")
                        xdma = nc.sync.dma_start(
                            out=xp_t[:],
                            in_=x[s, 128 * kc:128 * (kc + 1), r0:r0 + PIECE_ROWS, :].bitcast(f32r))
                        if p < 2:
                            gate = y_dmas[-3 if p == 0 else -1]
                            _br.add_dep_helper(xdma.ins, gate.ins, reason="pace x behind y")
                        xps.append(xp_t)
                    xpieces[p] = xps
                    # fp8 copies of x for the trans DoubleRow matmuls; piece 0
                    # converts per 4-row third so the first dw chunk is not
                    # gated on whole-piece conversions at startup
                    x8s = [px8.tile([128, 2, PIECE_ROWS, W], f8, tag="x8", name="x8")
                           for s in range(2)]

                    def convert(s, kc, a, n, i):
                        if i % 2 == 0:
                            nc.vector.tensor_scalar(out=x8s[s][:, kc, a:a + n, :],
                                                    in0=xps[2 * s + kc][:, a:a + n, :],
                                                    scalar1=1.0, scalar2=None, op0=MULT)
                        else:
                            nc.scalar.activation(out=x8s[s][:, kc, a:a + n, :],
                                                 in_=xps[2 * s + kc][:, a:a + n, :],
                                                 func=Copy, scale=1.0)

                    if p > 0:
                        for i, (s, kc) in enumerate(((0, 0), (1, 1), (0, 1), (1, 0))):
                            convert(s, kc, 0, PIECE_ROWS, i)
                    for third in range(PIECE_ROWS // CH_ROWS):
                        rr = third * CH_ROWS
                        if p == 0:
                            for i, (s, kc) in enumerate(((0, 0), (1, 1), (0, 1), (1, 0))):
                                convert(s, kc, rr, CH_ROWS, i + third)
                        pt = psB.tile([128, CH_ROWS, W], f32, tag="ptrans", name="ptrans")
                        for s in range(2):
                            nc.tensor.matmul(pt[:], wtr8_sb[s][:],
                                             x8s[s][:, :, rr:rr + CH_ROWS, :],
                                             start=(s == 0), stop=(s == 1),
                                             perf_mode=DR)
                        xin_dst = xin[:, PAD + r0 + rr:PAD + r0 + rr + CH_ROWS, PAD:PAD + W]
                        if (3 * p + third) % 2 == 0:
                            nc.vector.tensor_scalar(out=xin_dst, in0=pt[:],
                                                    scalar1=1.0 / 64.0, scalar2=btr_sb[:],
                                                    op0=MULT, op1=ADD)
                        else:
                            nc.scalar.activation(out=xin_dst, in_=pt[:], func=Ident,
                                                 bias=btr_sb[:], scale=1.0 / 64.0)

                ost = {}
                fstore = {}

                def emit_dw(c):
                    r0 = CH_ROWS * c
                    base = (PAD + r0) * WP + PAD
                    Ffo = [pF.tile([128, 2, CH_ROWS, W], f8, tag="ffo", name=f"ffo{s}")
                           for s in range(2)]

                    def copy_pair(i, dst_a, src_a, dst_b, src_b):
                        # alternate engines; emit right after the source psum
                        # group stops so its bank recycles ASAP
                        if i % 2 == 0:
                            nc.vector.tensor_scalar(out=dst_a, in0=src_a,
                                                    scalar1=SF / S2, scalar2=None,
                                                    op0=MULT)
                            nc.scalar.activation(out=dst_b, in_=src_b, func=Copy,
                                                 scale=SF / S2)
                        else:
                            nc.scalar.activation(out=dst_a, in_=src_a, func=Copy,
                                                 scale=SF / S2)
                            nc.vector.tensor_scalar(out=dst_b, in0=src_b,
                                                    scalar1=SF / S2, scalar2=None,
                                                    op0=MULT)

                    def one_dil(d):
                        v = 1 if d == 3 else 0   # swapped bank for d=3
                        fp = psF.tile([128, CH_ROWS, W], f32, tag="fps", name="fps")
                        pairs = (TAP_PAIRS[:1] if mode == "nodw" else TAP_PAIRS)
                        for pi, (ta, tb) in enumerate(pairs):
                            rhs = xin_ap(base + _tap_off(ta, d),
                                         [[_tap_off(tb, d) - _tap_off(ta, d), 2],
                                          [WP, CH_ROWS], [1, W]])
                            lhsT = w2_ap(v, ta * 128, [[(tb - ta) * 128, 2], [1, 128]])
                            nc.tensor.matmul(fp[:], lhsT, rhs, start=(pi == 0),
                                             stop=(pi == len(pairs) - 1),
                                             perf_mode=DR)
                        # partition-aligned extractions -> fp8 f tiles (x SF/S2)
                        if d == 1:
                            copy_pair(0, Ffo[0][0:64, 0], fp[0:64],
                                      Ffo[1][64:128, 0], fp[64:128])
                        elif d == 3:   # swapped: s1 in low half
                            copy_pair(1, Ffo[1][0:64, 0], fp[0:64],
                                      Ffo[0][64:128, 0], fp[64:128])
                        else:
                            copy_pair(0, Ffo[0][0:64, 1], fp[0:64],
                                      Ffo[1][64:128, 1], fp[64:128])

                    def part1():
                        one_dil(1)
                        one_dil(3)

                    def part2():
                        one_dil(5)
                        # center tap (shared by all dilations), swapped bank
                        fc = psF.tile([128, CH_ROWS, W], f32, tag="fps", name="fpc")
                        nc.tensor.matmul(fc[:], w2b[1][:, 4, :],
                                         xin_ap(base, [[WP, CH_ROWS], [1, W]]),
                                         start=True, stop=True)
                        copy_pair(1, Ffo[1][0:64, 1], fc[0:64],
                                  Ffo[0][64:128, 1], fc[64:128])
                        fstore[c] = Ffo
                    return part1, part2

                def emit_fo(c, samples=(0, 1)):
                    r0 = CH_ROWS * c
                    Ffo = fstore[c]
                    p, rr0 = c // (PIECE_ROWS // CH_ROWS), (c % (PIECE_ROWS // CH_ROWS)) * CH_ROWS
                    xps = xpieces[p]
                    if c % 4 == 0 and samples[0] == 0:
                        for key in ((0, 0), (0, 1), (1, 0), (1, 1)):
                            ost[key] = pO.tile([128, OST_ROWS, W], f16, tag="ost", name="ost")
                    for s in samples:
                        for mj in range(2):
                            po = psO.tile([128, CH_ROWS, W], f32, tag="ops", name="ops")
                            if mode == "nofo":
                                nc.tensor.matmul(po[:], wfox_sb[0][:, 128 * mj:128 * (mj + 1)],
                                                 xps[2 * s][:, rr0:rr0 + CH_ROWS, :],
                                                 start=True, stop=True)
                            else:
                                for kc in range(2):
                                    nc.tensor.matmul(po[:], wfox_sb[kc][:, 128 * mj:128 * (mj + 1)],
                                                     xps[2 * s + kc][:, rr0:rr0 + CH_ROWS, :],
                                                     start=(kc == 0), stop=False)
                                nc.tensor.matmul(po[:],
                                                 wfoF_sb[s][:, :, 128 * mj:128 * (mj + 1)],
                                                 Ffo[s][:], start=False, stop=True,
                                                 perf_mode=DR)
                            stg = ost[(s, mj)]
                            rr = (c % 4) * CH_ROWS
                            if (s + mj) % 2 == 0:
                                nc.vector.tensor_scalar(out=stg[:, rr:rr + CH_ROWS, :], in0=po[:],
                                                        scalar1=1.0 / G, scalar2=bfo_sb[mj][:],
                                                        op0=MULT, op1=ADD)
                            else:
                                nc.scalar.activation(out=stg[:, rr:rr + CH_ROWS, :], in_=po[:],
                                                     func=Ident, bias=bfo_sb[mj][:],
                                                     scale=1.0 / G)
                    if samples[-1] != 1:
                        return
                    if c >= N_CHUNKS - 4 and c % 2 == 1:
                        # tail: flush every 8 rows so the last DMA starts sooner
                        rr = ((c % 4) // 2) * (OST_ROWS // 2)
                        for s in range(2):
                            for mj in range(2):
                                nc.gpsimd.dma_start(
                                    out=o[s, 128 * mj:128 * (mj + 1), r0 + CH_ROWS - OST_ROWS // 2:r0 + CH_ROWS, :],
                                    in_=ost[(s, mj)][:, rr:rr + OST_ROWS // 2, :])
                    elif c % 4 == 3:
                        for s in range(2):
                            for mj in range(2):
                                nc.gpsimd.dma_start(
                                    out=o[s, 128 * mj:128 * (mj + 1), r0 + CH_ROWS - OST_ROWS:r0 + CH_ROWS, :],
                                    in_=ost[(s, mj)][:])

                # pieces of 3 chunks; chunks lag one piece; fo of chunk c-1
                # interleaves between the dilation halves of dw chunk c
                emitted_dw = 0
                emitted_fo = 0

                def step_dw(c):
                    p1, p2 = emit_dw(c)
                    p1()
                    if c >= 1:
                        emit_fo(c - 1, samples=(0,))
                    p2()
                    if c >= 1:
                        emit_fo(c - 1, samples=(1,))
                        fstore.pop(c - 1)

                for p in range(N_PIECES):
                    emit_piece(p)
                    hi = 3 * p + 1  # chunk c needs xin rows through 4c+9
                    while emitted_dw < hi:
                        step_dw(emitted_dw)
                        emitted_dw += 1
                while emitted_dw < N_CHUNKS:
                    step_dw(emitted_dw)
                    emitted_dw += 1
                emit_fo(N_CHUNKS - 1)
                fstore.pop(N_CHUNKS - 1)

            if repeat == 1:
                body()
            else:
                with tc.For_i(0, repeat, 1):
                    body()

    nc.compile()
    return nc


def _prep_weights(w_gk, b_gk, w_tr, b_tr, w_fi, b_fi, w_fo, b_fo):
    import ml_dtypes
    f32 = np.float32
    f8 = ml_dtypes.float8_e4m3
    wtr = np.zeros((2, 2, 128, 128), f32)
    wgk = np.zeros((2, 2, 128, 128), f32)
    for kc in range(2):
        blkT = w_tr[:, 128 * kc:128 * (kc + 1)].T  # [128 in, 64 mid]
        blkG = w_gk[:, 128 * kc:128 * (kc + 1)].T
        for s in range(2):
            wtr[kc, s, :, 64 * s:64 * (s + 1)] = blkT
            wgk[kc, s, :, 64 * s:64 * (s + 1)] = blkG
    wfi2 = np.zeros((2, 128, 128), f32)
    wfi2[0, 0:64, 0:64] = w_fi.T * S2
    wfi2[0, 64:128, 64:128] = w_fi.T * S2
    wfi2[1, 0:64, 64:128] = w_fi.T * S2
    wfi2[1, 64:128, 0:64] = w_fi.T * S2
    # fuse_outside: cat = [x(0:256), f1(256:320), f3(320:384), f5(384:448)]
    wfox = np.zeros((2, 128, 256), f32)
    for kc in range(2):
        wfox[kc] = w_fo[:, 128 * kc:128 * (kc + 1)].T * G
    b1 = w_fo[:, 256:320].T   # [64, 256]
    b3 = w_fo[:, 320:384].T
    b5 = w_fo[:, 384:448].T
    bc = b1 + b3 + b5
    wfoF = np.zeros((2, 128, 2, 256), f32)
    wfoF[0, 0:64, 0] = b1
    wfoF[0, 64:128, 0] = b3
    wfoF[0, 0:64, 1] = b5
    wfoF[0, 64:128, 1] = bc
    wfoF[1, 0:64, 0] = b3
    wfoF[1, 64:128, 0] = b1
    wfoF[1, 0:64, 1] = bc
    wfoF[1, 64:128, 1] = b5
    wfoF = (wfoF * (G / SF)).astype(f8)
    wtr8 = np.zeros((2, 128, 2, 128), f32)
    for s in range(2):
        for kc in range(2):
            wtr8[s, :, kc, 64 * s:64 * (s + 1)] = w_tr[:, 128 * kc:128 * (kc + 1)].T * 64.0
    wtr8 = wtr8.astype(f8)
    btr = np.tile(b_tr, 2).reshape(128, 1).astype(f32)
    bgk = np.tile(b_gk, 2).reshape(128, 1).astype(f32)
    bfo_t = b_fo + w_fo[:, 256:448] @ np.tile(b_fi, 3)
    bfo = bfo_t.reshape(2, 128, 1).astype(f32)
    return dict(wtr=wtr, wgk=wgk, wfi2=wfi2, wfox=wfox, wfoF=wfoF, wtr8=wtr8,
                btr=btr, bgk=bgk, bfo=bfo)


def _get_nc(repeat=1, mode="full"):
    key = ("nc", repeat, mode)
    if key not in _CACHE:
        _CACHE[key] = _build(repeat, mode)
    return _CACHE[key]


def _in_maps(x, y, wd):
    in_maps = []
    for c in range(N_CORES):
        m = dict(wd)
        m["x"] = np.ascontiguousarray(x[SPC * c:SPC * (c + 1)])
        m["y"] = np.ascontiguousarray(y[SPC * c:SPC * (c + 1)])
        in_maps.append(m)
    return in_maps


def kernel(x, y, w_gk, b_gk, w_tr, b_tr, w_fi, b_fi, w_fo, b_fo):
    from concourse.bass_utils import run_bass_kernel_spmd

    nc = _get_nc(1)
    wd = _prep_weights(
        np.asarray(w_gk, np.float32), np.asarray(b_gk, np.float32),
        np.asarray(w_tr, np.float32), np.asarray(b_tr, np.float32),
        np.asarray(w_fi, np.float32), np.asarray(b_fi, np.float32),
        np.asarray(w_fo, np.float32), np.asarray(b_fo, np.float32))
    in_maps = _in_maps(np.asarray(x, np.float32), np.asarray(y, np.float32), wd)
    res = run_bass_kernel_spmd(nc, in_maps, core_ids=list(range(N_CORES)))
    out = np.concatenate([np.asarray(res.results[c]["o"], np.float32)
                          for c in range(N_CORES)], axis=0)
    return out


# ---------------- timing (dev-only; not used by the grader) ----------------

def _make_callable(nc):
    import jax
    import concourse.mybir as mybir
    from concourse.bass2jax import _bass_exec_p, partition_id_tensor
    from jax.sharding import Mesh, PartitionSpec
    from jax.experimental.shard_map import shard_map

    in_names, out_names, out_avals = [], [], []
    for alloc in nc.m.functions[0].allocations:
        if not isinstance(alloc, mybir.MemoryLocationSet):
            continue
        name = alloc.memorylocations[0].name
        if alloc.kind == "ExternalInput":
            if nc.partition_id_tensor is None or name != nc.partition_id_tensor.name:
                in_names.append(name)
        elif alloc.kind == "ExternalOutput":
            out_names.append(name)
            out_avals.append(jax.core.ShapedArray(tuple(alloc.tensor_shape),
                                                  mybir.dt.np(alloc.dtype)))
    n_params = len(in_names)
    all_in = list(in_names) + list(out_names)
    part = nc.partition_id_tensor.name if nc.partition_id_tensor else None
    if part:
        all_in.append(part)

    def _body(*args):
        operands = list(args)
        if part:
            operands.append(partition_id_tensor())
        outs = _bass_exec_p.bind(
            *operands, out_avals=tuple(out_avals), in_names=tuple(all_in),
            out_names=tuple(out_names), lowering_input_output_aliases=(),
            sim_require_finite=True, sim_require_nnan=True, nc=nc)
        return tuple(outs)

    devices = jax.devices()[:N_CORES]
    mesh = Mesh(np.asarray(devices), ("core",))
    nin = n_params + len(out_names)
    fn = jax.jit(shard_map(_body, mesh=mesh, in_specs=(PartitionSpec("core"),) * nin,
                           out_specs=(PartitionSpec("core"),) * len(out_names),
                           check_rep=False), keep_unused=True)
    return fn, in_names, out_names, out_avals


def _prep_fn(repeat, in_maps, mode="full"):
    import jax
    nc = _get_nc(repeat, mode)
    fn, in_names, out_names, out_avals = _make_callable(nc)
    concat_in = []
    for n in in_names:
        per = [np.asarray(in_maps[c][n]) for c in range(N_CORES)]
        concat_in.append(np.concatenate(per, axis=0))
    zeros = [np.zeros((N_CORES * a.shape[0], *a.shape[1:]), a.dtype) for a in out_avals]
    dev_in = [jax.device_put(a) for a in concat_in] + [jax.device_put(z) for z in zeros]
    return fn, dev_in


def _time_pair(in_maps, R=33, rounds=16, mode="full"):
    """Interleaved timing of the R=1 and R=R variants so host/tunnel drift
    cancels. Returns (t1_min, tR_min)."""
    import jax, time
    fn1, in1 = _prep_fn(1, in_maps, mode)
    fnR, inR = _prep_fn(R, in_maps, mode)
    for _ in range(3):
        jax.block_until_ready(fn1(*in1))
        jax.block_until_ready(fnR(*inR))
    t1s, tRs = [], []
    for _ in range(rounds):
        t0 = time.perf_counter()
        jax.block_until_ready(fn1(*in1))
        t1s.append(time.perf_counter() - t0)
        t0 = time.perf_counter()
        jax.block_until_ready(fnR(*inR))
        tRs.append(time.perf_counter() - t0)
    return min(t1s), min(tRs)


def measure_exec_ns(R=33, trials=16, mode="full"):
    rng = np.random.default_rng(0)
    wd = _prep_weights(
        rng.standard_normal((64, 256)).astype(np.float32) * 0.06,
        rng.standard_normal(64).astype(np.float32) * 0.06,
        rng.standard_normal((64, 256)).astype(np.float32) * 0.06,
        rng.standard_normal(64).astype(np.float32) * 0.06,
        rng.standard_normal((64, 64)).astype(np.float32) * 0.12,
        rng.standard_normal(64).astype(np.float32) * 0.12,
        rng.standard_normal((256, 448)).astype(np.float32) * 0.05,
        rng.standard_normal(256).astype(np.float32) * 0.05)
    x = rng.standard_normal((B, IN_C, H, W)).astype(np.float32)
    y = rng.standard_normal((B, IN_C, H, W)).astype(np.float32)
    in_maps = _in_maps(x, y, wd)
    t1, tR = _time_pair(in_maps, R=R, rounds=trials, mode=mode)
    per_iter = (tR - t1) / (R - 1)
    print(f"t1={t1*1e3:.3f} ms  t{R}={tR*1e3:.3f} ms  per-iter={per_iter*1e6:.1f} us")
    return per_iter * 1e9
